# revision 1
# baseline (speedup 1.0000x reference)
"""BeitSelfAttention block-sparse attention kernel for 8 Trainium2 NeuronCores.

Strategy (data-parallel over batch, B=8 -> one batch element per core):
  - Host pre-transposes hidden states (hsT [768,1569] bf16 per core) and
    pre-gathers the relative-position bias as exp(bias)*multiplicity tables
    (index math only; all FLOPs stay on device).
  - Device per core: QKV projections on PE (bf16, fp32 psum accumulate),
    producing qT/kT in [d, token] layout and V in token-major pair tiles.
  - Block-sparse attention computed TRANSPOSED: per key-block-pair p (98 keys)
    and head h, scores simT = kT_pair^T @ qT[:, attending-query-cols] go to
    PSUM; softmax = exp on ACT (no max subtraction needed: logits are bounded
    small) * exp(bias) on DVE; AV uses V_pair as the stationary operand with a
    ones-column rider that accumulates the softmax denominator, accumulating
    outT[65, 1569] per head in PSUM across pairs.  The cls token is handled by
    a dense cls-key row (block-diag kT trick) and by including query-column 0
    in every pair's column list.
  - Normalize rows by the accumulated denominator (DVE recip + GPSIMD
    partition broadcast + DVE mult) and DMA out [12, 64, 1569] per core.
  - Host reassembles [8, 1569, 768].
"""

import os
from contextlib import ExitStack

import numpy as np

NCLS, BS, NBLK, NPAIR, NH, DH = 1, 49, 32, 16, 12, 64
B, S, D = 8, 1569, 768
NTOK = S - NCLS  # 1568
SCALE = 0.125
N_CORES = 8
SPAD = 1632  # kT/hsT padded width so 128-col stationary slices stay in bounds


# ----------------------------------------------------------------------------
# host-side layout
# ----------------------------------------------------------------------------

def _build_layout(rand_idx):
    rand_idx = np.asarray(rand_idx)
    mult = np.zeros((NBLK, NBLK), np.int32)
    for m in range(NBLK):
        for o in (-1, 0, 1):
            mult[m, (m + o) % NBLK] += 1
        for r in rand_idx[m]:
            mult[m, int(r)] += 1

    segs = []
    gcol = 0  # global packed column across banks
    for p in range(NPAIR):
        att = sorted(set(np.nonzero(mult[:, 2 * p])[0]) | set(np.nonzero(mult[:, 2 * p + 1])[0]))
        cols = {0}
        for m in att:
            cols.update(range(1 + BS * m, 1 + BS * (m + 1)))
        cols = sorted(cols)
        runs = []
        c0 = cols[0]
        prev = cols[0]
        for c in cols[1:]:
            if c != prev + 1:
                runs.append((c0, prev - c0 + 1))
                c0 = c
            prev = c
        runs.append((c0, prev - c0 + 1))
        cur = None
        for (rc, rw) in runs:
            while rw > 0:
                take = min(rw, 512 - (gcol % 512))
                if cur is None or cur["bank"] != gcol // 512:
                    cur = {"p": p, "runs": [], "width": 0,
                           "bank": gcol // 512, "off": gcol % 512}
                    segs.append(cur)
                cur["runs"].append((rc, take))
                cur["width"] += take
                gcol += take
                rc += take
                rw -= take
                if gcol % 512 == 0:
                    cur = None
        cur = None  # next pair starts a new segment

    nbank = (gcol + 511) // 512
    banks = [512] * (gcol // 512) + ([gcol % 512] if gcol % 512 else [])
    ng = (nbank + 1) // 2
    if nbank < ng * 2:  # odd bank count: synthesize an empty filler bank
        segs.append({"p": 0, "runs": [], "width": 0, "bank": nbank, "off": 0})
        banks.append(0)
        nbank += 1
    # pad-fill every bank to 512 written columns (score matmuls over dummy
    # query cols; ebias is 0 there) so exp never reads stale psum bytes
    last_in_bank = {}
    for i, sg in enumerate(segs):
        if sg["bank"] not in last_in_bank or sg["off"] >= segs[last_in_bank[sg["bank"]]]["off"]:
            last_in_bank[sg["bank"]] = i
    for bk, used in enumerate(banks):
        segs[last_in_bank[bk]]["pad_w"] = 512 - used
    for sg in segs:
        sg.setdefault("pad_w", 0)
        sg["acol"] = sg["bank"] * 512 + sg["off"]
        sg["g"] = sg["bank"] // 2
        sg["goff"] = (sg["bank"] % 2) * 512 + sg["off"]

    segs.sort(key=lambda s: (s["g"], s["bank"], s["off"]))
    groups = [[] for _ in range(ng)]
    for sg in segs:
        groups[sg["g"]].append(sg)

    # AV runs: outT lives as four per-bank quarter tiles [65, <=512].  Split
    # score runs at 512-col quarter boundaries AND at already-written/fresh
    # column transitions (PSUM has_written semantics); tag the first matmul
    # per quarter.
    touched = [False] * 4
    written = np.zeros(S, bool)
    for sg in segs:
        av = []
        oc = 0
        for (rc, rw) in sg["runs"]:
            c, w = rc, rw
            while w > 0:
                bnd = ((c // 512) + 1) * 512
                take = min(w, bnd - c)
                sub0 = c
                while sub0 < c + take:
                    st = bool(written[sub0])
                    sub1 = sub0
                    while sub1 < c + take and bool(written[sub1]) == st:
                        sub1 += 1
                    bnk = sub0 // 512
                    av.append({"qc0": sub0, "w": sub1 - sub0,
                               "oc": oc + (sub0 - c), "first": not touched[bnk]})
                    touched[bnk] = True
                    sub0 = sub1
                written[c:c + take] = True
                oc += take
                c += take
                w -= take
        sg["av_runs"] = av

    gocc = [max(0, min(1024, gcol - g * 1024)) for g in range(ng)]
    last_touch = [0] * 4
    for sg in segs:
        for av in sg["av_runs"]:
            last_touch[av["qc0"] // 512] = max(last_touch[av["qc0"] // 512], sg["g"])
    return {"segs": segs, "groups": groups, "mult": mult, "NBANK": nbank,
            "NG": ng, "last_touch": last_touch, "gocc": gocc}


def _build_ebias(lay, rel_table, rel_pos_index):
    mult = lay["mult"]
    ng = lay["NG"]
    eb = np.zeros((NH, 98, ng * 1024), np.float32)
    for sg in lay["segs"]:
        p = sg["p"]
        ktok = 1 + 98 * p + np.arange(98)
        kblk = 2 * p + np.arange(98) // BS
        acol = sg["acol"]
        for (rc, rw) in sg["runs"]:
            qtok = np.arange(rc, rc + rw)
            qblk = np.maximum(qtok - 1, 0) // BS
            m = mult[qblk][:, kblk].T.astype(np.float32)  # [98, rw]
            m[:, qtok == 0] = 1.0
            idx = rel_pos_index[qtok[:, None], ktok[None, :]]  # [rw, 98]
            val = rel_table[idx]  # [rw, 98, NH]
            ebv = np.exp(val.astype(np.float32)) * m.T[:, :, None]
            eb[:, :, acol:acol + rw] = ebv.transpose(2, 1, 0)
            acol += rw
    return eb


def _build_ebias_cls(rel_table, rel_pos_index):
    idx = rel_pos_index[np.arange(S), 0]
    return np.exp(rel_table[idx].astype(np.float32)).T.copy()  # [NH, S]


# ----------------------------------------------------------------------------
# walrus workaround: split the TileContext tail drain's sem waits
# ----------------------------------------------------------------------------

def _patch_tile_drain():
    import concourse.tile as tile
    from concourse.vector_clock import ScopedClock, VectorClock

    if getattr(tile.TileContext, "_beit_drain_patch", False):
        return

    def _drain_and_barrier(self, tick_clock, wait_clock):
        gc_vec = tick_clock.global_clock
        n = len(gc_vec)
        nonzero = [i for i in range(n) if gc_vec[i] > 0] or [0]
        for i in range(0, len(nonzero), 1):
            chunk = set(nonzero[i:i + 1])
            vec = VectorClock([gc_vec[j] if j in chunk else 0 for j in range(n)])
            drain_inst = self.nc.sync.drain()
            wait_clock.add_sem_waits(drain_inst.ins, ScopedClock({None: vec}))
        self.nc.all_engine_barrier()
        assert self.sems is not None
        popped = self.nc._tile_sem_poison_stack.pop()
        assert popped is self._sem_poison
        self.nc.clear_and_free_semaphores(list(self.sems.allocated().values()))
        self.nc.all_engine_barrier()

    tile.TileContext._drain_and_barrier = _drain_and_barrier
    tile.TileContext._beit_drain_patch = True


def _split_excess_waits(nc, mybir, limit=1):
    """This walrus build allows very few sem waits per instruction; move the
    excess onto EventSemaphore carrier instructions inserted just before."""
    ctr = [0]
    for f in nc.m.functions:
        for bb in f.blocks:
            il = bb.instructions
            out = []
            for inst in il:
                si = inst.sync_info
                if si is not None and si.on_wait and len(si.on_wait) > limit:
                    waits = list(si.on_wait)
                    over = waits[limit:]
                    for j in range(0, len(over), limit):
                        ctr[0] += 1
                        ev = mybir.InstEventSemaphore(
                            name=f"WSPLIT-{ctr[0]}", ins=[], outs=[],
                            engine=inst.engine,
                            sync_info=mybir.SyncInfo(on_wait=over[j:j + limit],
                                                     on_update=[]),
                        )
                        nc.register_instruction(ev, overwrite=True)
                        out.append(ev)
                    si.on_wait = waits[:limit]
                out.append(inst)
            il[:] = out
    return ctr[0]


# ----------------------------------------------------------------------------
# device kernel emission
# ----------------------------------------------------------------------------

def _emit(nc, tile, mybir, lay):
    import concourse.bass as bass

    bf = mybir.dt.bfloat16
    f32 = mybir.dt.float32
    ng = lay["NG"]

    hsT_d = nc.dram_tensor("hsT", [D, S], bf, kind="ExternalInput")
    wq_d = nc.dram_tensor("Wq", [D, D], bf, kind="ExternalInput")
    wk_d = nc.dram_tensor("Wk", [D, D], bf, kind="ExternalInput")
    wv_d = nc.dram_tensor("Wv", [D, D], bf, kind="ExternalInput")
    bq_d = nc.dram_tensor("bq_row", [1, D], bf, kind="ExternalInput")
    bv_d = nc.dram_tensor("bv_row", [1, D], bf, kind="ExternalInput")
    eb_d = nc.dram_tensor("ebias", [NH, 98, ng * 1024], bf, kind="ExternalInput")
    ebc_d = nc.dram_tensor("ebias_cls", [NH, S], bf, kind="ExternalInput")
    bdo_d = nc.dram_tensor("bd_ones", [NH, NH * 65 + 64], bf, kind="ExternalInput")
    out_d = nc.dram_tensor("out_t", [NH, DH, S], f32, kind="ExternalOutput")

    Exp = mybir.ActivationFunctionType.Exp
    s_chunks = [(0, 512), (512, 512), (1024, 512), (1536, S - 1536)]

    with tile.TileContext(nc) as tc, ExitStack() as ctx:
        consts = ctx.enter_context(tc.tile_pool(name="consts", bufs=1))
        persist = ctx.enter_context(tc.tile_pool(name="persist", bufs=1))

        ones_row = consts.tile([1, S], bf, tag="ones", name="ones")
        nc.vector.memset(ones_row[:, :], 1.0)
        bq_sb = consts.tile([1, D], bf, tag="bq", name="bq")
        nc.sync.dma_start(out=bq_sb[:, :], in_=bq_d[:, :])
        bv_sb = consts.tile([1, D], bf, tag="bv", name="bv")
        nc.sync.dma_start(out=bv_sb[:, :], in_=bv_d[:, :])

        qT = [persist.tile([128, S], bf, tag=f"qT{t}", name=f"qT{t}") for t in range(6)]
        kT = [persist.tile([128, SPAD], bf, tag=f"kT{t}", name=f"kT{t}") for t in range(6)]
        for t in range(6):
            nc.vector.memset(kT[t][:, S:SPAD], 0.0)
        vst = persist.tile([98, NPAIR * NH * 65 + 64], bf, tag="vst", name="vst")
        nc.vector.memset(vst[:, NPAIR * NH * 65:], 0.0)
        bdv = persist.tile([NH, NH * 65 + 64], bf, tag="bdv", name="bdv")
        bdk = persist.tile([128, 6, NH], bf, tag="bdk", name="bdk")
        atc = persist.tile([NH, S], bf, tag="aTcls", name="aTcls")
        ebc_sb = persist.tile([NH, S], bf, tag="ebc", name="ebc")
        nc.sync.dma_start(out=ebc_sb[:, :], in_=ebc_d[:, :])
        nc.sync.dma_start(out=bdv[:, :], in_=bdo_d[:, :])

        # ---------------- phase A: projections ----------------
        with tc.tile_pool(name="phA", bufs=1) as phA, \
             tc.tile_pool(name="pp", bufs=2, space="PSUM") as pp, \
             tc.tile_pool(name="stg", bufs=2) as stg:
            # just-in-time DMA ordering: interleave the W/hsT tiles the first
            # projection chains need, and defer Wk/Wv loads until used
            hsT = []
            w_sb = {"q": [], "k": [], "v": []}
            for t in range(6):
                wt = phA.tile([128, D], bf, tag=f"wq{t}", name=f"wq{t}")
                nc.sync.dma_start(out=wt[:, :], in_=wq_d[t * 128:(t + 1) * 128, :])
                w_sb["q"].append(wt)
                hst = phA.tile([128, SPAD], bf, tag=f"hsT{t}", name=f"hsT{t}")
                nc.sync.dma_start(out=hst[:, 0:S], in_=hsT_d[t * 128:(t + 1) * 128, :])
                nc.vector.memset(hst[:, S:SPAD], 0.0)
                hsT.append(hst)

            def load_w(nm, dram):
                for t in range(6):
                    wt = phA.tile([128, D], bf, tag=f"w{nm}{t}", name=f"w{nm}{t}")
                    nc.gpsimd.dma_start(out=wt[:, :], in_=dram[t * 128:(t + 1) * 128, :])
                    w_sb[nm].append(wt)

            # qT / kT projections: out tiles [128 dims, S]
            for name, wts, dst, has_bias in (("q", w_sb["q"], qT, True),
                                             ("k", w_sb["k"], kT, False)):
                if name == "k":
                    load_w("k", wk_d)
                    wts = w_sb["k"]
                for dt in range(6):
                    for (c0, cw) in s_chunks:
                        ps = pp.tile([128, 512], f32, tag="pq", name="pq")
                        for kt in range(6):
                            nc.tensor.matmul(
                                ps[:, :cw],
                                lhsT=wts[kt][:, dt * 128:(dt + 1) * 128],
                                rhs=hsT[kt][:, c0:c0 + cw],
                                start=(kt == 0),
                                stop=(kt == 5 and not has_bias),
                            )
                        if has_bias:
                            nc.tensor.matmul(
                                ps[:, :cw],
                                lhsT=bq_sb[0:1, dt * 128:(dt + 1) * 128],
                                rhs=ones_row[0:1, c0:c0 + cw],
                                start=False, stop=True,
                            )
                            nc.any.tensor_scalar_mul(dst[dt][:, c0:c0 + cw], ps[:, :cw], SCALE)
                        else:
                            nc.any.tensor_copy(dst[dt][:, c0:c0 + cw], ps[:, :cw])

            # ones columns of the augmented V store
            load_w("v", wv_d)
            vst4 = vst[:, 0:NPAIR * NH * 65].rearrange("a (p h e) -> a p h e", p=NPAIR, h=NH)
            nc.vector.memset(vst4[:, :, :, 64:65], 1.0)

            # V projection in 98-token pair chunks (tokens 1..1568),
            # M padded to 128 for fast weight load
            for p in range(NPAIR):
                c0 = 1 + 98 * p
                ps = pp.tile([128, D], f32, tag="pv", name="pv")
                for (h0, hw) in ((0, 512), (512, 256)):
                    for kt in range(6):
                        nc.tensor.matmul(
                            ps[:, h0:h0 + hw],
                            lhsT=hsT[kt][:, c0:c0 + 128],
                            rhs=w_sb["v"][kt][:, h0:h0 + hw],
                            start=(kt == 0), stop=False,
                        )
                    nc.tensor.matmul(
                        ps[:, h0:h0 + hw],
                        lhsT=ones_row[0:1, 0:128],
                        rhs=bv_sb[0:1, h0:h0 + hw],
                        start=False, stop=True,
                    )
                dst = vst4[:, p, :, 0:64]
                src = ps[0:98, :].rearrange("a (h e) -> a h e", h=NH)
                nc.any.tensor_copy(dst, src)

            # cls-token V row -> block-diag v_cls (bdv) via tiny scatter DMAs
            ps = pp.tile([128, D], f32, tag="pv", name="pv")
            for (h0, hw) in ((0, 512), (512, 256)):
                for kt in range(6):
                    nc.tensor.matmul(
                        ps[0:1, h0:h0 + hw],
                        lhsT=hsT[kt][:, 0:1],
                        rhs=w_sb["v"][kt][:, h0:h0 + hw],
                        start=(kt == 0), stop=False,
                    )
                nc.tensor.matmul(
                    ps[0:1, h0:h0 + hw],
                    lhsT=ones_row[0:1, 0:1],
                    rhs=bv_sb[0:1, h0:h0 + hw],
                    start=False, stop=True,
                )
            vcls_sb = stg.tile([1, D], bf, tag="vcls", name="vcls")
            nc.any.tensor_copy(vcls_sb[:, :], ps[0:1, :])
            for h in range(NH):
                nc.sync.dma_start(
                    out=bdv[h:h + 1, h * 65:h * 65 + 64],
                    in_=vcls_sb[0:1, h * 64:(h + 1) * 64],
                )

            # block-diag cls-key columns of kT
            nc.vector.memset(bdk[:, :, :], 0.0)
            for t in range(6):
                for half in range(2):
                    r0 = half * 64
                    nc.vector.tensor_copy(
                        bdk[r0:r0 + 64, t, 2 * t + half:2 * t + half + 1],
                        kT[t][r0:r0 + 64, 0:1],
                    )

        # ---------------- cls-key row: scores + exp ----------------
        with tc.tile_pool(name="clsps", bufs=1, space="PSUM") as clsps, \
             tc.tile_pool(name="stg2", bufs=1) as stg2:
            cls_ps = clsps.tile([NH, S], f32, tag="clsps", name="clsps")
            for (c0, cw) in s_chunks:
                for t in range(6):
                    nc.tensor.matmul(
                        cls_ps[:, c0:c0 + cw],
                        lhsT=bdk[:, t, :],
                        rhs=qT[t][:, c0:c0 + cw],
                        start=(t == 0), stop=(t == 5),
                    )
            clsraw = stg2.tile([NH, S], bf, tag="clsraw", name="clsraw")
            nc.scalar.activation(clsraw[:, :], cls_ps[:, :], Exp)
            nc.vector.tensor_mul(atc[:, :], clsraw[:, :], ebc_sb[:, :])

        # ---------------- phase B: block-sparse attention per head ----------
        with tc.tile_pool(name="scps", bufs=2, space="PSUM") as scps, \
             tc.tile_pool(name="otps", bufs=1, space="PSUM") as otps, \
             tc.tile_pool(name="ab", bufs=4) as ab, \
             tc.tile_pool(name="ebp", bufs=8) as ebp, \
             tc.tile_pool(name="drp", bufs=2, space="DRAM") as drp, \
             tc.tile_pool(name="nrm", bufs=3) as nrm:
            quarters = [(0, 512), (512, 512), (1024, 512), (1536, S - 1536)]

            def emit_av(h, g, aT, outTs):
                for sg in lay["groups"][g]:
                    vh = vst[0:98, sg["p"] * NH * 65 + h * 65:sg["p"] * NH * 65 + h * 65 + 128]
                    for av in sg["av_runs"]:
                        q = av["qc0"] // 512
                        lc = av["qc0"] - 512 * q
                        nc.tensor.matmul(
                            outTs[q][:, lc:lc + av["w"]],
                            lhsT=vh,
                            rhs=aT[0:98, sg["goff"] + av["oc"]:sg["goff"] + av["oc"] + av["w"]],
                            start=av["first"], stop=False,
                        )

            def emit_head_tail(h, q, outT):
                # cls-key AV (K=12 block-diag v_cls); closes this quarter's
                # psum bank accumulation group.  Then normalize + write out.
                qb, qw = quarters[q]
                nc.tensor.matmul(
                    outT[:, 0:qw],
                    lhsT=bdv[:, h * 65:h * 65 + 128],
                    rhs=atc[:, qb:qb + qw],
                    start=False, stop=True,
                )
                den = nrm.tile([65, 512], f32, tag="den", name="den")
                nc.vector.reciprocal(den[64:65, :qw], outT[64:65, :qw])
                den_dr = drp.tile([1, 512], f32, tag="dend", name="dend")
                nc.sync.dma_start(out=den_dr[:, :qw], in_=den[64:65, :qw])
                bc = nrm.tile([64, 512], f32, tag="bc", name="bc")
                src = den_dr[:, :qw]
                bcast = bass.AP(tensor=src.tensor, offset=src.offset,
                                ap=[[0, 64]] + [list(d) for d in src.ap][1:])
                nc.sync.dma_start(out=bc[:, :qw], in_=bcast)
                ob = nrm.tile([64, 512], f32, tag="ob", name="ob")
                nc.vector.tensor_mul(ob[:, :qw], outT[0:64, :qw], bc[:, :qw])
                nc.gpsimd.dma_start(out=out_d[h][:, qb:qb + qw], in_=ob[:, :qw])

            # software pipeline over (head, group) units with a one-unit skew
            # between the exp/mult producers and the consuming AV matmuls, so
            # the next group's score matmuls hide the ACT/DVE latency.
            outT_by_h = {}
            pending = None  # (h, g, aT)
            for h in range(NH):
                dt = h // 2
                r0 = (h % 2) * 64
                outT_by_h[h] = [
                    otps.tile([128, qw], f32, tag=f"outQ{q}", name=f"outQ{q}")
                    for q, (qb, qw) in enumerate(quarters)
                ]
                for g in range(ng):
                    sc = scps.tile([128, 1024], f32, tag="sc", name="sc")
                    for sg in lay["groups"][g]:
                        kc0 = 1 + 98 * sg["p"]
                        oc = 0
                        for (rc, rw) in sg["runs"]:
                            nc.tensor.matmul(
                                sc[:, sg["goff"] + oc:sg["goff"] + oc + rw],
                                lhsT=kT[dt][r0:r0 + 64, kc0:kc0 + 128],
                                rhs=qT[dt][r0:r0 + 64, rc:rc + rw],
                                start=True, stop=True,
                            )
                            oc += rw
                    gw = lay["gocc"][g]
                    eb_sb = ebp.tile([98, 1024], bf, tag="eb", name="eb")
                    eb_eng = nc.sync if g % 2 == 0 else nc.gpsimd
                    eb_eng.dma_start(out=eb_sb[:, :gw], in_=eb_d[h, :, g * 1024:g * 1024 + gw])
                    ar = ab.tile([98, 1024], bf, tag="ar", name="ar")
                    nc.scalar.activation(ar[:, :gw], sc[0:98, :gw], Exp)
                    aT = ab.tile([98, 1024], bf, tag="aT", name="aT")
                    nc.vector.tensor_mul(aT[:, :gw], ar[:, :gw], eb_sb[:, :gw])
                    if pending is not None:
                        ph, pg, paT = pending
                        emit_av(ph, pg, paT, outT_by_h[ph])
                        for q in range(4):
                            if lay["last_touch"][q] == pg:
                                emit_head_tail(ph, q, outT_by_h[ph][q])
                        if pg == ng - 1:
                            outT_by_h.pop(ph)
                    pending = (h, g, aT)
            ph, pg, paT = pending
            emit_av(ph, pg, paT, outT_by_h[ph])
            for q in range(4):
                if lay["last_touch"][q] == pg:
                    emit_head_tail(ph, q, outT_by_h[ph][q])
            outT_by_h.pop(ph)

    _split_excess_waits(nc, mybir, limit=1)
    return nc


def _bench_pjrt(nc, in_maps, n_cores, iters=20, warmup=3):
    """Time repeated executions of the compiled kernel (no donation; inputs
    stay device-resident).  Returns (per_iter_ns, results_list)."""
    import time

    import jax
    import numpy as np
    from jax.sharding import Mesh, PartitionSpec
    from jax.experimental.shard_map import shard_map

    from concourse import mybir
    from concourse.bass2jax import (_bass_exec_p, install_neuronx_cc_hook,
                                    partition_id_tensor)

    install_neuronx_cc_hook()
    partition_name = nc.partition_id_tensor.name if nc.partition_id_tensor else None
    in_names, out_names, out_avals, zero_outs = [], [], [], []
    for alloc in nc.m.functions[0].allocations:
        if not isinstance(alloc, mybir.MemoryLocationSet):
            continue
        name = alloc.memorylocations[0].name
        if alloc.kind == "ExternalInput":
            if name != partition_name:
                in_names.append(name)
        elif alloc.kind == "ExternalOutput":
            shape = tuple(alloc.tensor_shape)
            dtype = mybir.dt.np(alloc.dtype)
            out_names.append(name)
            out_avals.append(jax.core.ShapedArray(shape, dtype))
            zero_outs.append(np.zeros(shape, dtype))
    n_params = len(in_names)
    all_in_names = in_names + out_names + ([partition_name] if partition_name else [])

    def _body(*args):
        operands = list(args)
        if partition_name is not None:
            operands.append(partition_id_tensor())
        return tuple(_bass_exec_p.bind(
            *operands,
            out_avals=tuple(out_avals),
            in_names=tuple(all_in_names),
            out_names=tuple(out_names),
            lowering_input_output_aliases=(),
            sim_require_finite=True,
            sim_require_nnan=True,
            nc=nc,
        ))

    devices = jax.devices()[:n_cores]
    mesh = Mesh(np.asarray(devices), ("core",))
    n_outs = len(out_names)
    sharded = jax.jit(
        shard_map(_body, mesh=mesh,
                  in_specs=(PartitionSpec("core"),) * (n_params + n_outs),
                  out_specs=(PartitionSpec("core"),) * n_outs,
                  check_rep=False),
        keep_unused=True,
    )
    per_core = [[np.asarray(m[name]) for name in in_names] for m in in_maps]
    concat_in = [np.concatenate([per_core[c][i] for c in range(n_cores)], axis=0)
                 for i in range(n_params)]
    concat_zeros = [np.zeros((n_cores * z.shape[0], *z.shape[1:]), z.dtype)
                    for z in zero_outs]
    dev_in = [jax.device_put(a) for a in concat_in + concat_zeros]
    out = sharded(*dev_in)
    jax.block_until_ready(out)
    for _ in range(warmup):
        out = sharded(*dev_in)
    jax.block_until_ready(out)
    t0 = time.perf_counter()
    for _ in range(iters):
        out = sharded(*dev_in)
    jax.block_until_ready(out)
    dt = (time.perf_counter() - t0) / iters
    results = [
        {name: np.asarray(out[i]).reshape(n_cores, *out_avals[i].shape)[c]
         for i, name in enumerate(out_names)}
        for c in range(n_cores)
    ]
    return int(dt * 1e9), results


# ----------------------------------------------------------------------------
# public entry point
# ----------------------------------------------------------------------------

def kernel(hidden_states, Wq, bq, Wk, Wv, bv, rel_table, rel_pos_index, rand_idx):
    import ml_dtypes

    import concourse.bass as bass
    import concourse.tile as tile
    from concourse import mybir
    from concourse.bass_utils import run_bass_kernel_spmd

    _patch_tile_drain()
    bf16 = ml_dtypes.bfloat16

    hidden_states = np.asarray(hidden_states, np.float32)
    Wq = np.asarray(Wq, np.float32)
    Wk = np.asarray(Wk, np.float32)
    Wv = np.asarray(Wv, np.float32)
    bq = np.asarray(bq, np.float32)
    bv = np.asarray(bv, np.float32)
    rel_table = np.asarray(rel_table, np.float32)
    rel_pos_index = np.asarray(rel_pos_index)
    rand_idx = np.asarray(rand_idx)

    lay = _build_layout(rand_idx)
    eb = _build_ebias(lay, rel_table, rel_pos_index).astype(bf16)
    ebc = _build_ebias_cls(rel_table, rel_pos_index).astype(bf16)
    bdo = np.zeros((NH, NH * 65 + 64), np.float32)
    for h in range(NH):
        bdo[h, h * 65 + 64] = 1.0
    bdo = bdo.astype(bf16)

    shared = {
        "Wq": Wq.astype(bf16), "Wk": Wk.astype(bf16), "Wv": Wv.astype(bf16),
        "bq_row": bq.reshape(1, D).astype(bf16),
        "bv_row": bv.reshape(1, D).astype(bf16),
        "ebias": eb, "ebias_cls": ebc, "bd_ones": bdo,
    }
    in_maps = []
    for b in range(B):
        m = dict(shared)
        m["hsT"] = np.ascontiguousarray(hidden_states[b].T).astype(bf16)
        in_maps.append(m)

    nc = bass.Bass()
    _emit(nc, tile, mybir, lay)

    kernel.last_nc = nc
    kernel.last_in_maps = in_maps
    bench_iters = int(os.environ.get("BEIT_BENCH", "0"))
    if bench_iters > 0:
        per_iter_ns, results = _bench_pjrt(nc, in_maps, N_CORES, iters=bench_iters)
        kernel.last_exec_time_ns = per_iter_ns
    else:
        res = run_bass_kernel_spmd(nc, in_maps, core_ids=list(range(N_CORES)))
        results = res.results

    out = np.empty((B, S, NH * DH), np.float32)
    for b in range(B):
        o = results[b]["out_t"]  # [NH, DH, S]
        out[b] = o.transpose(2, 0, 1).reshape(S, NH * DH)
    return out



# revision 19
# speedup vs baseline: 1.1182x; 1.1182x over previous
"""BeitSelfAttention block-sparse attention kernel for 8 Trainium2 NeuronCores.

Strategy (data-parallel over batch, B=8 -> one batch element per core):
  - fp8e4 DoubleRow matmuls for QKV projections and block-sparse scores
    (two 128-row k-tiles per pass at 0.5 cycles/row).
  - The relative-position bias AND the block-sparsity mask are host-packed
    into one fp8 table (16x scale) and added into the score PSUM by an
    identity DoubleRow matmul; gather multiplicity (rand/local block
    collisions) is realized by a few extra AV matmuls against half-masked V
    copies, so no per-element multiply is needed on DVE at all.
  - The cls KEY rides as a 99th score row per key-pair tile (designated to
    one pair per query via the bias mask); the cls QUERY is packed column 0.
  - AV is computed transposed: out[q, dh] psum tiles [128 queries, 65] with a
    ones-rider column in V accumulating the softmax denominator per query
    IN THE FREE DIM, so normalization is a per-partition reciprocal +
    scaled copy (no cross-partition broadcast, no DRAM round trip).
  - Output written as [S, 768] fp32 - already the final layout.
"""

import os
from contextlib import ExitStack

import numpy as np

NCLS, BS, NBLK, NPAIR, NH, DH = 1, 49, 32, 16, 12, 64
B, S, D = 8, 1569, 768
NTOK = S - NCLS  # 1568
N_CORES = 8
KEYS = 99            # 98 pair keys + 1 cls row
NSLOT = 16           # AV psum slots (2 blocks each, partitions 0-48 / 64-112)
CLSB = 1024 + 2 * 65  # cls-query corner col in the AV psum tile (bank 2)
AVW = CLSB + 65      # used width of the AV psum tile
SP8 = 1856           # per-dt stride of qT8/kT8 (zero strip beyond data)
KCOLS = NPAIR * KEYS  # 1584 pair-major key columns
HS8W = 1664          # hs8 padded token width
QZ = 1576            # qT8 ktile1 landing col (zeros)
KZ = 1584            # kT8 ktile1 landing col (zeros)
OPEN_V = 1e-15       # opener operand value; den init = 2*OPEN_V^2


# ----------------------------------------------------------------------------
# host-side layout
# ----------------------------------------------------------------------------

def _slot_target(qtok):
    """Map a query token to its AV psum target: (partition0, col_base).
    Slot s holds block 2s at partitions 0-48 and block 2s+1 at 64-112;
    the cls query (token 0) lives at partition 0 of the cls corner."""
    if qtok == 0:
        return 0, CLSB
    qb = (qtok - 1) // BS
    s = qb // 2
    cb = 65 * s if s < 7 else (512 + 65 * (s - 7) if s < 14 else 1024 + 65 * (s - 14))
    return 64 * (qb % 2) + (qtok - 1 - BS * qb), cb


def _build_layout(rand_idx):
    rand_idx = np.asarray(rand_idx)
    mult = np.zeros((NBLK, NBLK), np.int32)
    for m in range(NBLK):
        for o in (-1, 0, 1):
            mult[m, (m + o) % NBLK] += 1
        for r in rand_idx[m]:
            mult[m, int(r)] += 1

    # pack attending query columns per key-pair into banks of 512 (groups of
    # 1024). Block units (the cls col, or a 49-token block) never straddle a
    # bank boundary: the packing pads to the bank edge instead, so every AV
    # piece starts at an aligned psum partition (0 or 64). Pad columns carry
    # no scores; the bias matmul assigns them the -448 mask.
    segs = []
    gcol = 0
    for p in range(NPAIR):
        att = sorted(set(np.nonzero(mult[:, 2 * p])[0])
                     | set(np.nonzero(mult[:, 2 * p + 1])[0]))
        units = [(0, 1)] + [(1 + BS * m, BS) for m in att]
        cur = None
        prev_end = None
        for (uc, uw) in units:
            if 512 - (gcol % 512) < uw:
                gcol += 512 - (gcol % 512)  # pad to bank edge
                cur = None
            if cur is None or cur["bank"] != gcol // 512:
                cur = {"p": p, "runs": [], "bank": gcol // 512,
                       "off": gcol % 512}
                segs.append(cur)
                prev_end = None
            if prev_end == uc:
                rc0, rw0 = cur["runs"][-1]
                cur["runs"][-1] = (rc0, rw0 + uw)
            else:
                cur["runs"].append((uc, uw))
            prev_end = uc + uw
            gcol += uw
        cur = None  # next pair starts a new segment

    nbank = (gcol + 511) // 512
    ng = (nbank + 1) // 2
    for sg in segs:
        sg["g"] = sg["bank"] // 2
        sg["goff"] = (sg["bank"] % 2) * 512 + sg["off"]
    # group occupancy (incl. pad columns): all banks full except the last
    last_bank_fill = gcol - (nbank - 1) * 512
    gocc = []
    for g in range(ng):
        b0, b1 = 2 * g, 2 * g + 1
        occ = 0
        for b in (b0, b1):
            if b < nbank - 1:
                occ += 512
            elif b == nbank - 1:
                occ += last_bank_fill
        gocc.append(occ)

    # per-group score pieces (runs split to <=256), start flag per 512-region
    score_pieces = [[] for _ in range(ng)]
    bank_started = [False] * nbank
    for sg in segs:
        oc = 0
        for (rc, rw) in sg["runs"]:
            c, w = rc, rw
            while w > 0:
                take = min(w, 256)
                st = not bank_started[sg["bank"]]
                bank_started[sg["bank"]] = True
                score_pieces[sg["g"]].append(
                    (sg["p"], c, take, sg["goff"] + oc, st))
                oc += take
                c += take
                w -= take

    # bias pieces per group: cover each bank's occupancy in <=256 chunks,
    # split at the pad watermark (scores wrote [0, used); pads [used, bw) are
    # still pending-zero and must be covered by their own assign piece);
    # last chunk per bank carries stop
    bank_used = [0] * nbank
    for sg in segs:
        w = sum(rw for (_, rw) in sg["runs"])
        bank_used[sg["bank"]] = max(bank_used[sg["bank"]], sg["off"] + w)
    bias_pieces = [[] for _ in range(ng)]
    for g in range(ng):
        for half in range(2):
            b = 2 * g + half
            if b >= nbank:
                continue
            bw = 512 if b < nbank - 1 else last_bank_fill
            used = bank_used[b]
            c = 0
            while c < bw:
                lim = used if c < used else bw
                take = min(256, lim - c)
                bias_pieces[g].append(
                    (half * 512 + c, take, c + take >= bw))
                c += take

    # AV pieces per group: runs split at block units -> aligned partitions
    av_pieces = [[] for _ in range(ng)]
    for sg in segs:
        oc = 0
        for (rc, rw) in sg["runs"]:
            c, w = rc, rw
            while w > 0:
                take = 1 if c == 0 else min(w, BS - (c - 1) % BS)
                qp0, cb = _slot_target(c)
                av_pieces[sg["g"]].append(
                    ("vst", sg["p"], sg["goff"] + oc, take, qp0, cb))
                oc += take
                c += take
                w -= take

    # per-group column -> (qtok, pair) maps (qtok -1 = pad), cls designation
    lb_cols = []
    for g in range(ng):
        qtok = np.full(1024, -1, np.int64)
        pair = np.zeros(1024, np.int64)
        lb_cols.append((qtok, pair))
    cls_seen = np.zeros(S, bool)
    cls_des = np.zeros((ng, 1024), bool)
    for sg in segs:
        qtok, pair = lb_cols[sg["g"]]
        oc = sg["goff"]
        for (rc, rw) in sg["runs"]:
            qtok[oc:oc + rw] = np.arange(rc, rc + rw)
            pair[oc:oc + rw] = sg["p"]
            fresh = ~cls_seen[rc:rc + rw]
            cls_des[sg["g"], oc:oc + rw] = fresh
            cls_seen[rc:rc + rw] = True
            oc += rw

    # multiplicity extras: (qb, kb) with mult >= 2 -> (m-1) extra AV matmuls
    # against a half-masked V copy (vste slot per distinct (pair, half))
    vste_cases = []      # (pair, half)
    col_of = {}
    for sg in segs:
        oc = sg["goff"]
        for (rc, rw) in sg["runs"]:
            for i in range(rw):
                col_of[(sg["p"], rc + i)] = (sg["g"], oc + i)
            oc += rw
    for qb in range(NBLK):
        for kb in range(NBLK):
            m = int(mult[qb, kb])
            if m < 2:
                continue
            p, half = kb // 2, kb % 2
            if (p, half) not in vste_cases:
                vste_cases.append((p, half))
            e = vste_cases.index((p, half))
            t0 = 1 + BS * qb
            g, oc = col_of[(p, t0)]
            qp0, cb = _slot_target(t0)
            for _ in range(m - 1):
                av_pieces[g].append(("vste", e, oc, BS, qp0, cb))
    return {"segs": segs, "mult": mult, "ng": ng, "gocc": gocc,
            "nbank": nbank, "score_pieces": score_pieces,
            "bias_pieces": bias_pieces, "av_pieces": av_pieces,
            "lb_cols": lb_cols, "cls_des": cls_des,
            "vste_cases": vste_cases}


def _build_bias8(lay, rel_table, rel_pos_index, f8np):
    """lb8 [NH, ng, 50, 2048]: rows (p, i) -> key 49i+p (cls at (49,1)),
    values 16*bias, -240 where masked."""
    ng = lay["ng"]
    mult = lay["mult"]
    MASK = -240.0
    lb = np.full((NH, ng, 50, 2, 1024), MASK, np.float32)
    for sg in lay["segs"]:
        g = sg["g"]
        p = sg["p"]
        ktok = 1 + 98 * p + np.arange(98)          # [98]
        kblk = 2 * p + (np.arange(98) // BS)
        oc = sg["goff"]
        for (rc, rw) in sg["runs"]:
            qtok = np.arange(rc, rc + rw)
            qblk = np.maximum(qtok - 1, 0) // BS
            att = (mult[qblk][:, kblk] > 0) | (qtok == 0)[:, None]  # [rw, 98]
            idx = rel_pos_index[qtok[:, None], ktok[None, :]]       # [rw, 98]
            val = 16.0 * rel_table[idx]                             # [rw,98,NH]
            val = np.where(att[:, :, None], np.clip(val, -200, 200), MASK)
            v = val.transpose(2, 1, 0)                              # [NH,98,rw]
            lb[:, g, 0:49, 0, oc:oc + rw] = v[:, 0:49]
            lb[:, g, 0:49, 1, oc:oc + rw] = v[:, 49:98]
            # cls row: designated pair only
            des = lay["cls_des"][g, oc:oc + rw]
            cidx = rel_pos_index[qtok, 0]
            cval = np.clip(16.0 * rel_table[cidx], -200, 200)       # [rw, NH]
            cv = np.where(des[:, None], cval, MASK).T               # [NH, rw]
            lb[:, g, 49, 1, oc:oc + rw] = cv
            oc += rw
    return lb.reshape(NH, ng, 50, 2048).astype(f8np)


# ----------------------------------------------------------------------------
# walrus workaround: split the TileContext tail drain's sem waits
# ----------------------------------------------------------------------------

def _patch_tile_drain():
    import concourse.tile as tile
    from concourse.vector_clock import ScopedClock, VectorClock

    if getattr(tile.TileContext, "_beit_drain_patch", False):
        return

    def _drain_and_barrier(self, tick_clock, wait_clock):
        gc_vec = tick_clock.global_clock
        n = len(gc_vec)
        nonzero = [i for i in range(n) if gc_vec[i] > 0] or [0]
        for i in range(0, len(nonzero), 1):
            chunk = set(nonzero[i:i + 1])
            vec = VectorClock([gc_vec[j] if j in chunk else 0 for j in range(n)])
            drain_inst = self.nc.sync.drain()
            wait_clock.add_sem_waits(drain_inst.ins, ScopedClock({None: vec}))
        self.nc.all_engine_barrier()
        assert self.sems is not None
        popped = self.nc._tile_sem_poison_stack.pop()
        assert popped is self._sem_poison
        self.nc.clear_and_free_semaphores(list(self.sems.allocated().values()))
        self.nc.all_engine_barrier()

    tile.TileContext._drain_and_barrier = _drain_and_barrier
    tile.TileContext._beit_drain_patch = True


def _split_excess_waits(nc, mybir, limit=1):
    """This walrus build allows very few sem waits per instruction; move the
    excess onto EventSemaphore carrier instructions inserted just before."""
    ctr = [0]
    for f in nc.m.functions:
        for bb in f.blocks:
            il = bb.instructions
            out = []
            for inst in il:
                si = inst.sync_info
                if si is not None and si.on_wait and len(si.on_wait) > limit:
                    waits = list(si.on_wait)
                    over = waits[limit:]
                    for j in range(0, len(over), limit):
                        ctr[0] += 1
                        ev = mybir.InstEventSemaphore(
                            name=f"WSPLIT-{ctr[0]}", ins=[], outs=[],
                            engine=inst.engine,
                            sync_info=mybir.SyncInfo(on_wait=over[j:j + limit],
                                                     on_update=[]),
                        )
                        nc.register_instruction(ev, overwrite=True)
                        out.append(ev)
                    si.on_wait = waits[:limit]
                out.append(inst)
            il[:] = out
    return ctr[0]


# ----------------------------------------------------------------------------
# device kernel emission
# ----------------------------------------------------------------------------

def _emit(nc, tile, mybir, lay):
    import concourse.bass as bass

    bf = mybir.dt.bfloat16
    f8 = mybir.dt.float8e4
    f32 = mybir.dt.float32
    DR = mybir.MatmulPerfMode.DoubleRow
    Exp = mybir.ActivationFunctionType.Exp
    ng = lay["ng"]
    NE = max(1, len(lay["vste_cases"]))

    hs8_d = nc.dram_tensor("hs8", [3, 128, 2 * HS8W], f8, kind="ExternalInput")
    hsk8_d = nc.dram_tensor("hsk8", [3, 128, 2 * KCOLS], f8, kind="ExternalInput")
    wq8_d = nc.dram_tensor("Wq8", [3, 128, 2 * D], f8, kind="ExternalInput")
    wk8_d = nc.dram_tensor("Wk8", [3, 128, 2 * D], f8, kind="ExternalInput")
    wv8_d = nc.dram_tensor("Wv8", [3, 128, 2 * D], f8, kind="ExternalInput")
    bq8_d = nc.dram_tensor("bq8", [1, 1024], f8, kind="ExternalInput")
    bv8_d = nc.dram_tensor("bv8", [1, 1024], f8, kind="ExternalInput")
    i8_d = nc.dram_tensor("I8", [50, 2 * KEYS], f8, kind="ExternalInput")
    cz16_d = nc.dram_tensor("cz16", [1, 2048], bf, kind="ExternalInput")
    lb8_d = nc.dram_tensor("lb8", [NH, ng, 50, 2048], f8, kind="ExternalInput")
    out_d = nc.dram_tensor("out_s", [S, D], f32, kind="ExternalOutput")

    def ap3(sl, s1, n1, s2, n2):
        return bass.AP(tensor=sl.tensor, offset=sl.offset,
                       ap=[list(sl.ap[0]), [s1, n1], [s2, n2]])

    def slot_col(s):
        return 65 * s if s < 7 else (512 + 65 * (s - 7) if s < 14
                                     else 1024 + 65 * (s - 14))

    av_banks = [(0, 455), (512, 455), (1024, AVW - 1024)]

    with tile.TileContext(nc) as tc, ExitStack() as ctx:
        consts = ctx.enter_context(tc.tile_pool(name="consts", bufs=1))
        persist = ctx.enter_context(tc.tile_pool(name="persist", bufs=1))

        c64 = consts.tile([1, 1024], f8, tag="c64", name="c64")
        nc.vector.memset(c64[:, :], 1.0 / 64.0)
        o8c = consts.tile([1, 1024], f8, tag="o8c", name="o8c")
        nc.vector.memset(o8c[:, 0:512], 1.0 / 64.0)
        nc.vector.memset(o8c[:, 512:1024], 0.0)
        z16 = consts.tile([1, 128], bf, tag="z16", name="z16")
        nc.vector.memset(z16[:, :], OPEN_V)
        o16 = consts.tile([1, 512], bf, tag="o16", name="o16")
        nc.vector.memset(o16[:, :], OPEN_V)
        bq8 = consts.tile([1, 1024], f8, tag="bq8", name="bq8")
        nc.gpsimd.dma_start(out=bq8[:, :], in_=bq8_d[:, :])
        bv8 = consts.tile([1, 1024], f8, tag="bv8", name="bv8")
        nc.gpsimd.dma_start(out=bv8[:, :], in_=bv8_d[:, :])
        i8sb = consts.tile([50, 2 * KEYS], f8, tag="i8", name="i8")
        nc.gpsimd.dma_start(out=i8sb[:, :], in_=i8_d[:, :])

        qT8 = persist.tile([128, 6 * SP8], f8, tag="qT8", name="qT8")
        kT8 = persist.tile([128, 6 * SP8], f8, tag="kT8", name="kT8")
        for t in range(6):
            nc.vector.memset(qT8[:, t * SP8 + S:(t + 1) * SP8], 0.0)
            nc.vector.memset(kT8[:, t * SP8 + KCOLS:(t + 1) * SP8], 0.0)
        vst = persist.tile([KEYS, NPAIR * NH * 65], bf, tag="vst", name="vst")
        vst4 = vst[:, :].rearrange("a (p h e) -> a p h e", p=NPAIR, h=NH)
        nc.vector.memset(vst4[:, :, :, 64:65], 1.0)
        vste = persist.tile([KEYS, NE * NH * 65], bf, tag="vste", name="vste")
        nc.gpsimd.memset(vste[:, :], 0.0)
        vste4 = vste[:, :].rearrange("a (e h c) -> a e h c", e=NE, h=NH)
        def bcast49(dram_sl, inner):
            # DRAM source broadcast across 49 partitions
            return bass.AP(tensor=dram_sl.tensor, offset=dram_sl.offset,
                           ap=[[0, 49]] + inner)
        outS = persist.tile([128, (NSLOT + 1) * D], f32, tag="outS", name="outS")

        # ---------------- phase A: projections ----------------
        with tc.tile_pool(name="phA", bufs=1) as phA, \
             tc.tile_pool(name="pp", bufs=3, space="PSUM") as pp, \
             tc.tile_pool(name="ppv", bufs=2, space="PSUM") as ppv, \
             tc.tile_pool(name="stg", bufs=2) as stg:
            hs8, hsk8 = [], []
            w_sb = {"q": [], "k": [], "v": []}
            for j in range(3):
                t = phA.tile([128, 2 * HS8W], f8, tag=f"hs8_{j}", name=f"hs8_{j}")
                nc.sync.dma_start(out=t[:, :], in_=hs8_d[j])
                hs8.append(t)
                t = phA.tile([128, 2 * D], f8, tag=f"wq8_{j}", name=f"wq8_{j}")
                nc.sync.dma_start(out=t[:, :], in_=wq8_d[j])
                w_sb["q"].append(t)
            for j in range(3):
                t = phA.tile([128, 2 * KCOLS], f8, tag=f"hsk8_{j}", name=f"hsk8_{j}")
                nc.gpsimd.dma_start(out=t[:, :], in_=hsk8_d[j])
                hsk8.append(t)
            for nm, dram in (("k", wk8_d), ("v", wv8_d)):
                for j in range(3):
                    t = phA.tile([128, 2 * D], f8, tag=f"w{nm}8_{j}", name=f"w{nm}8_{j}")
                    nc.gpsimd.dma_start(out=t[:, :], in_=dram[j])
                    w_sb[nm].append(t)

            # qT8 / kT8 projections: psum [128 dims, chunk]
            qchunks = [(0, 512), (512, 512), (1024, 512), (1536, S - 1536)]
            kchunks = [(0, 512), (512, 512), (1024, 512), (1536, KCOLS - 1536)]
            for name, hsrc, hw_, chunks, dst, scale in (
                    ("q", hs8, HS8W, qchunks, qT8, 0.5),
                    ("k", hsk8, KCOLS, kchunks, kT8, 0.25)):
                for dt in range(6):
                    for (c0, cw) in chunks:
                        ps = pp.tile([128, 512], f32, tag="pq", name="pq")
                        first = True
                        for j in range(3):
                            lhsT = ap3(w_sb[name][j][:, dt * 128:dt * 128 + 1],
                                       D, 2, 1, 128)
                            s0 = 0
                            while s0 < cw:
                                sw = min(256, cw - s0)
                                rhs = ap3(hsrc[j][:, c0 + s0:c0 + s0 + 1],
                                          hw_, 2, 1, sw)
                                nc.tensor.matmul(ps[:, s0:s0 + sw], lhsT=lhsT,
                                                 rhs=rhs, start=first,
                                                 stop=False, perf_mode=DR)
                                first = False
                                s0 += sw
                        if name == "q":
                            s0 = 0
                            while s0 < cw:
                                sw = min(256, cw - s0)
                                lhsT = ap3(bq8[0:1, dt * 128:dt * 128 + 1],
                                           D - dt * 128, 2, 1, 128)
                                rhs = ap3(c64[0:1, 0:1], 512, 2, 1, sw)
                                nc.tensor.matmul(ps[:, s0:s0 + sw], lhsT=lhsT,
                                                 rhs=rhs, start=False,
                                                 stop=(s0 + sw >= cw),
                                                 perf_mode=DR)
                                s0 += sw
                        else:
                            # close the group with a zero-product DR matmul
                            lhsT = ap3(o8c[0:1, 512:513], 256, 2, 1, 128)
                            rhs = ap3(c64[0:1, 0:1], 512, 2, 1, cw)
                            nc.tensor.matmul(ps[:, 0:cw], lhsT=lhsT, rhs=rhs,
                                             start=False, stop=True,
                                             perf_mode=DR)
                        nc.vector.tensor_scalar_mul(
                            dst[:, dt * SP8 + c0:dt * SP8 + c0 + cw],
                            ps[:, 0:cw], scale)

            # V projection per pair: psum [128 tokens, 768]
            ecase = {pc: e for e, pc in enumerate(lay["vste_cases"])}
            for p in range(NPAIR):
                c0 = 1 + 98 * p
                ps = ppv.tile([128, D], f32, tag="pv", name="pv")
                for (h0, hw_) in ((0, 256), (256, 256), (512, 256)):
                    first = True
                    for j in range(3):
                        lhsT = ap3(hs8[j][:, c0:c0 + 1], HS8W, 2, 1, 128)
                        rhs = ap3(w_sb["v"][j][:, h0:h0 + 1], D, 2, 1, hw_)
                        nc.tensor.matmul(ps[:, h0:h0 + hw_], lhsT=lhsT,
                                         rhs=rhs, start=first, stop=False,
                                         perf_mode=DR)
                        first = False
                    lhsT = ap3(o8c[0:1, 0:1], 512, 2, 1, 128)
                    rhs = ap3(bv8[0:1, h0:h0 + 1], D - h0, 2, 1, hw_)
                    nc.tensor.matmul(ps[:, h0:h0 + hw_], lhsT=lhsT, rhs=rhs,
                                     start=False, stop=True, perf_mode=DR)
                src = ps[0:98, :].rearrange("a (h e) -> a h e", h=NH)
                nc.gpsimd.tensor_copy(vst4[0:98, p, :, 0:64], src)
                for half in range(2):
                    if (p, half) not in ecase:
                        continue
                    e = ecase[(p, half)]
                    nc.gpsimd.tensor_copy(vste4[0:98, e, :, 0:64], src)
                    if half == 0:
                        # zero the inactive upper half, rider=1 on lower
                        nc.sync.dma_start(out=vste4[49:98, e, :, 0:65],
                                          in_=bcast49(cz16_d[0:1, 0:1],
                                                      [[65, NH], [1, 65]]))
                        nc.gpsimd.memset(vste4[0:49, e, :, 64:65], 1.0)
                    else:
                        # zero the inactive lower half, rider=1 on upper
                        nc.gpsimd.memset(vste4[0:49, e, :, 0:65], 0.0)
                        nc.sync.dma_start(
                            out=vste4[49:98, e, :, 64:65],
                            in_=bcast49(cz16_d[0:1, 1024:1025],
                                        [[1, NH], [1, 1]]))

            # cls-token V row -> vst row 98 of every pair
            ps = ppv.tile([128, D], f32, tag="pv", name="pv")
            for (h0, hw_) in ((0, 256), (256, 256), (512, 256)):
                first = True
                for j in range(3):
                    lhsT = ap3(hs8[j][:, 0:1], HS8W, 2, 1, 1)
                    rhs = ap3(w_sb["v"][j][:, h0:h0 + 1], D, 2, 1, hw_)
                    nc.tensor.matmul(ps[0:1, h0:h0 + hw_], lhsT=lhsT, rhs=rhs,
                                     start=first, stop=False, perf_mode=DR)
                    first = False
                lhsT = ap3(o8c[0:1, 0:1], 512, 2, 1, 1)
                rhs = ap3(bv8[0:1, h0:h0 + 1], D - h0, 2, 1, hw_)
                nc.tensor.matmul(ps[0:1, h0:h0 + hw_], lhsT=lhsT, rhs=rhs,
                                 start=False, stop=True, perf_mode=DR)
            vcls = stg.tile([1, D], bf, tag="vcls", name="vcls")
            nc.vector.tensor_copy(vcls[:, :], ps[0:1, :])
            for p in range(NPAIR):
                nc.sync.dma_start(out=vst4[98:99, p, :, 0:64],
                                  in_=vcls[0:1, :].rearrange("a (h e) -> a h e", h=NH))

        # ---------------- phase B: block-sparse attention per head ----------
        with tc.tile_pool(name="scps", bufs=2, space="PSUM") as scps, \
             tc.tile_pool(name="avps", bufs=1, space="PSUM") as avps, \
             tc.tile_pool(name="ab", bufs=3) as ab, \
             tc.tile_pool(name="lbp", bufs=3) as lbp, \
             tc.tile_pool(name="nrm", bufs=2) as nrm:

            def emit_openers(avt):
                # init every used AV psum byte to a tiny value ((1/64)^2 * 2)
                for (b0, bw) in av_banks:
                    first = True
                    c = 0
                    while c < bw:
                        take = min(256, bw - c)
                        nc.tensor.matmul(
                            avt[:, b0 + c:b0 + c + take],
                            lhsT=ap3(c64[0:1, 0:1], 512, 2, 1, 128),
                            rhs=ap3(c64[0:1, 0:1], 512, 2, 1, take),
                            start=first, stop=False, perf_mode=DR)
                        first = False
                        c += take

            def emit_av(h, g, aT, avt):
                for (kind, pe, oc, w, qp0, cb) in lay["av_pieces"][g]:
                    if kind == "vst":
                        rhs = vst4[0:KEYS, pe, h, 0:65]
                    else:
                        rhs = vste4[0:KEYS, pe, h, 0:65]
                    nc.tensor.matmul(
                        avt[qp0:qp0 + w, cb:cb + 65],
                        lhsT=aT[0:KEYS, oc:oc + w], rhs=rhs,
                        start=False, stop=False)

            def emit_head_tail(h, avt):
                # 1-col closers: end each bank's group on all 128 partitions
                for (b0, bw) in av_banks:
                    nc.tensor.matmul(avt[:, b0:b0 + 1],
                                     lhsT=ap3(c64[0:1, 0:1], 512, 2, 1, 128),
                                     rhs=ap3(c64[0:1, 0:1], 512, 2, 1, 1),
                                     start=False, stop=True, perf_mode=DR)
                rcol = nrm.tile([128, 17], f32, tag="rcol", name="rcol")
                for (i0, i1, base, n) in ((0, 7, 64, 7),
                                          (7, 14, 512 + 64, 7),
                                          (14, 17, 1024 + 64, 3)):
                    d0 = avt[:, base:base + 1]
                    nc.vector.reciprocal(
                        rcol[:, i0:i1],
                        bass.AP(tensor=d0.tensor, offset=d0.offset,
                                ap=[list(d0.ap[0]), [65, n]]))
                for s in range(NSLOT):
                    eng = nc.vector if s % 2 == 0 else nc.gpsimd
                    eng.tensor_scalar_mul(
                        outS[:, s * D + h * DH:s * D + h * DH + DH],
                        avt[:, slot_col(s):slot_col(s) + 64],
                        rcol[:, s:s + 1])
                nc.vector.tensor_scalar_mul(
                    outS[0:1, NSLOT * D + h * DH:NSLOT * D + h * DH + DH],
                    avt[0:1, CLSB:CLSB + 64], rcol[0:1, 16:17])

            pending = None
            avt_by_h = {}
            for h in range(NH):
                dt, r0 = h // 2, 64 * (h % 2)
                avt = avps.tile([128, 1536], f32, tag="avt", name="avt")
                avt_by_h[h] = avt
                for g in range(ng):
                    gw = lay["gocc"][g]
                    sc = scps.tile([128, 1024], f32, tag="sc", name="sc")
                    lb = lbp.tile([50, 2048], f8, tag="lb", name="lb")
                    nc.sync.dma_start(out=lb[:, :], in_=lb8_d[h, g])
                    for (p, rc, rw, oc, st) in lay["score_pieces"][g]:
                        lhsT = ap3(kT8[r0:r0 + 64, dt * SP8 + 99 * p:dt * SP8 + 99 * p + 1],
                                   KZ - 99 * p, 2, 1, KEYS)
                        rhs = ap3(qT8[r0:r0 + 64, dt * SP8 + rc:dt * SP8 + rc + 1],
                                  QZ - rc, 2, 1, rw)
                        nc.tensor.matmul(sc[0:KEYS, oc:oc + rw], lhsT=lhsT,
                                         rhs=rhs, start=st, stop=False,
                                         perf_mode=DR)
                    for (bc0, bw, sp) in lay["bias_pieces"][g]:
                        lhsT = ap3(i8sb[0:50, 0:1], KEYS, 2, 1, KEYS)
                        rhs = ap3(lb[0:50, bc0:bc0 + 1], 1024, 2, 1, bw)
                        nc.tensor.matmul(sc[0:KEYS, bc0:bc0 + bw], lhsT=lhsT,
                                         rhs=rhs, start=False, stop=sp,
                                         perf_mode=DR)
                    aT = ab.tile([KEYS, 1024], bf, tag="aT", name="aT")
                    nc.scalar.activation(aT[:, 0:gw], sc[0:KEYS, 0:gw], Exp)
                    if pending is not None:
                        ph, pg, paT = pending
                        if pg == 0:
                            emit_openers(avt_by_h[ph])
                        emit_av(ph, pg, paT, avt_by_h[ph])
                        if pg == ng - 1:
                            emit_head_tail(ph, avt_by_h.pop(ph))
                    pending = (h, g, aT)
            ph, pg, paT = pending
            if pg == 0:
                emit_openers(avt_by_h[ph])
            emit_av(ph, pg, paT, avt_by_h[ph])
            emit_head_tail(ph, avt_by_h.pop(ph))

            # output DMA per slot half (+ cls token row)
            for s in range(NSLOT):
                t0 = 1 + 98 * s
                nc.gpsimd.dma_start(out=out_d[t0:t0 + 49, :],
                                    in_=outS[0:49, s * D:(s + 1) * D])
                nc.gpsimd.dma_start(out=out_d[t0 + 49:t0 + 98, :],
                                    in_=outS[64:113, s * D:(s + 1) * D])
            nc.gpsimd.dma_start(out=out_d[0:1, :],
                                in_=outS[0:1, NSLOT * D:(NSLOT + 1) * D])

    _split_excess_waits(nc, mybir, limit=1)
    return nc


# ----------------------------------------------------------------------------
# host-side input prep
# ----------------------------------------------------------------------------

def _prepare(hidden_states, Wq, bq, Wk, Wv, bv, rel_table, rel_pos_index, rand_idx):
    import ml_dtypes

    import concourse.bass as bass
    import concourse.tile as tile
    from concourse import mybir

    _patch_tile_drain()
    f8np = ml_dtypes.float8_e4m3

    hidden_states = np.asarray(hidden_states, np.float32)
    Wq = np.asarray(Wq, np.float32)
    Wk = np.asarray(Wk, np.float32)
    Wv = np.asarray(Wv, np.float32)
    bq = np.asarray(bq, np.float32)
    bv = np.asarray(bv, np.float32)
    rel_table = np.asarray(rel_table, np.float32)
    rel_pos_index = np.asarray(rel_pos_index)
    rand_idx = np.asarray(rand_idx)

    lay = _build_layout(rand_idx)
    lb8 = _build_bias8(lay, rel_table, rel_pos_index, f8np)

    def packW(Wm):
        # [3, 128, 2, 768]: (j, p, i, n) = W[256j + 128i + p, n] * 8
        return np.ascontiguousarray(
            (Wm.reshape(3, 2, 128, D).transpose(0, 2, 1, 3) * 8.0)
            .reshape(3, 128, 2 * D)).astype(f8np)

    i8 = np.zeros((50, 2, KEYS), np.float32)
    for p in range(49):
        i8[p, 0, p] = 1.0 / 16.0
        i8[p, 1, 49 + p] = 1.0 / 16.0
    i8[49, 1, 98] = 1.0 / 16.0

    shared = {
        "Wq8": packW(Wq), "Wk8": packW(Wk), "Wv8": packW(Wv),
        "bq8": np.concatenate([bq * 64.0, np.zeros(256, np.float32)]
                              ).reshape(1, 1024).astype(f8np),
        "bv8": np.concatenate([bv * 64.0, np.zeros(256, np.float32)]
                              ).reshape(1, 1024).astype(f8np),
        "I8": i8.reshape(50, 2 * KEYS).astype(f8np),
        "lb8": lb8,
        "cz16": np.concatenate([np.zeros(1024, np.float32),
                                np.ones(1024, np.float32)]
                               ).reshape(1, 2048).astype(ml_dtypes.bfloat16),
    }

    # pair-major token order for the k projection (cls duplicated per pair)
    korder = np.empty(KCOLS, np.int64)
    for p in range(NPAIR):
        korder[99 * p:99 * p + 98] = 1 + 98 * p + np.arange(98)
        korder[99 * p + 98] = 0

    in_maps = []
    for b in range(B):
        hsT = hidden_states[b].T  # [768, S]
        hp = np.zeros((3, 128, 2, HS8W), np.float32)
        hp[:, :, :, 0:S] = (hsT / 8.0).reshape(3, 2, 128, S).transpose(0, 2, 1, 3)
        hk = (hsT[:, korder] / 8.0).reshape(3, 2, 128, KCOLS).transpose(0, 2, 1, 3)
        m = dict(shared)
        m["hs8"] = np.ascontiguousarray(hp.reshape(3, 128, 2 * HS8W)).astype(f8np)
        m["hsk8"] = np.ascontiguousarray(hk.reshape(3, 128, 2 * KCOLS)).astype(f8np)
        in_maps.append(m)

    nc = bass.Bass()
    _emit(nc, tile, mybir, lay)
    return nc, in_maps


# ----------------------------------------------------------------------------
# optional PJRT repeat-bench (unused by default; kept from v1)
# ----------------------------------------------------------------------------

def _bench_pjrt(nc, in_maps, n_cores, iters=20, warmup=3):
    import time

    import jax
    from jax.sharding import Mesh, PartitionSpec
    from jax.experimental.shard_map import shard_map

    from concourse import mybir
    from concourse.bass2jax import (_bass_exec_p, install_neuronx_cc_hook,
                                    partition_id_tensor)

    install_neuronx_cc_hook()
    partition_name = nc.partition_id_tensor.name if nc.partition_id_tensor else None
    in_names, out_names, out_avals, zero_outs = [], [], [], []
    for alloc in nc.m.functions[0].allocations:
        if not isinstance(alloc, mybir.MemoryLocationSet):
            continue
        name = alloc.memorylocations[0].name
        if alloc.kind == "ExternalInput":
            if name != partition_name:
                in_names.append(name)
        elif alloc.kind == "ExternalOutput":
            shape = tuple(alloc.tensor_shape)
            dtype = mybir.dt.np(alloc.dtype)
            out_names.append(name)
            out_avals.append(jax.core.ShapedArray(shape, dtype))
            zero_outs.append(np.zeros(shape, dtype))
    n_params = len(in_names)
    all_in_names = in_names + out_names + ([partition_name] if partition_name else [])

    def _body(*args):
        operands = list(args)
        if partition_name is not None:
            operands.append(partition_id_tensor())
        return tuple(_bass_exec_p.bind(
            *operands,
            out_avals=tuple(out_avals),
            in_names=tuple(all_in_names),
            out_names=tuple(out_names),
            lowering_input_output_aliases=(),
            sim_require_finite=True,
            sim_require_nnan=True,
            nc=nc,
        ))

    devices = jax.devices()[:n_cores]
    mesh = Mesh(np.asarray(devices), ("core",))
    n_outs = len(out_names)
    sharded = jax.jit(
        shard_map(_body, mesh=mesh,
                  in_specs=(PartitionSpec("core"),) * (n_params + n_outs),
                  out_specs=(PartitionSpec("core"),) * n_outs,
                  check_rep=False),
        keep_unused=True,
    )
    per_core = [[np.asarray(m[name]) for name in in_names] for m in in_maps]
    concat_in = [np.concatenate([per_core[c][i] for c in range(n_cores)], axis=0)
                 for i in range(n_params)]
    concat_zeros = [np.zeros((n_cores * z.shape[0], *z.shape[1:]), z.dtype)
                    for z in zero_outs]
    dev_in = [jax.device_put(a) for a in concat_in + concat_zeros]
    out = sharded(*dev_in)
    jax.block_until_ready(out)
    for _ in range(warmup):
        out = sharded(*dev_in)
    jax.block_until_ready(out)
    t0 = time.perf_counter()
    for _ in range(iters):
        out = sharded(*dev_in)
    jax.block_until_ready(out)
    dt = (time.perf_counter() - t0) / iters
    results = [
        {name: np.asarray(out[i]).reshape(n_cores, *out_avals[i].shape)[c]
         for i, name in enumerate(out_names)}
        for c in range(n_cores)
    ]
    return int(dt * 1e9), results


# ----------------------------------------------------------------------------
# public entry point
# ----------------------------------------------------------------------------

def kernel(hidden_states, Wq, bq, Wk, Wv, bv, rel_table, rel_pos_index, rand_idx):
    from concourse.bass_utils import run_bass_kernel_spmd

    nc, in_maps = _prepare(hidden_states, Wq, bq, Wk, Wv, bv,
                           rel_table, rel_pos_index, rand_idx)

    kernel.last_nc = nc
    kernel.last_in_maps = in_maps
    bench_iters = int(os.environ.get("BEIT_BENCH", "0"))
    if bench_iters > 0:
        per_iter_ns, results = _bench_pjrt(nc, in_maps, N_CORES, iters=bench_iters)
        kernel.last_exec_time_ns = per_iter_ns
    else:
        res = run_bass_kernel_spmd(nc, in_maps, core_ids=list(range(N_CORES)))
        results = res.results

    out = np.empty((B, S, D), np.float32)
    for b in range(B):
        out[b] = results[b]["out_s"]
    return out
